# revision 6
# baseline (speedup 1.0000x reference)
"""Batch-parallel multi-head attention layer for 8 TRN2 NeuronCores.

Problem: nn_AttentionLayer (B=8, T=1024, D=1024, H=16, hd=64, rotary).
Strategy: pure data-parallel over batch (8 cores, one batch element each,
no collectives). Per core, the contraction dim always sits on partitions and
scores are kept TRANSPOSED ([keys, queries]) so softmax normalization folds
into an ones-column of V and no on-chip transposes are needed.

v3: one flat software-pipelined stream of 128 (pair, query-half, key-tile)
slots. Each slot: QK matmul -> exp (ACT) -> PV two slots later (so a stalled
PV never sits at the head of the in-order Tensor queue blocking ready work).
The V projection, the next pair's q/k projections + RoPE, and the deferred
softmax normalizations are "filler" units scheduled into slots; pair-0's PVs
are deferred until the corresponding V tiles exist (e tiles stashed in a deep
pool). All weights are preloaded with large contiguous DMAs (host pre-permutes
Wqkv pair-major); q0/k0 projections start chunk-by-chunk underneath the input
DMA so the exp stream begins ~10us in, not ~57us.

Layouts (per core):
  xT      [d, t]  bf16  (host-transposed x)
  wqk     [d, 16*128] bf16 (host: pair-major q|k column blocks)
  qkT     [j, t]  on-chip, RoPE applied via uint32-bitcast stream_shuffle
  v_big   [t, jt, h, 65] on-chip ([.., 64] = ones column -> softmax denom Z)
  S^T     [j_keys, i_half] mixed (h0|h1) in one [128,1024] PSUM tile
  O_aug^T [65, i_half] = v_aug^T @ E accumulated over key tiles (row 64 = Z)
  ocatT   [f, t]  = O^T * (1/Z bcast via col-tiled K=1 matmuls), bf16
  y       [t, e]  = ocatT^T @ woutT + bias
"""

import os
import sys
import numpy as np

try:
    import concourse.bass as bass  # noqa: F401
except ImportError:
    sys.path.insert(0, "/opt/trn_rl_repo")

import ml_dtypes
from contextlib import ExitStack

import concourse.bass as bass
import concourse.tile as tile
from concourse import bacc, mybir

BF16 = ml_dtypes.bfloat16

B, T, D = 8, 1024, 1024
H, HD = 16, 64
NP = H // 2          # head pairs
ND = D // 128        # contraction chunks
NT = T // 128        # t tiles
THETA = 10000.0

F32 = mybir.dt.float32
DTB = mybir.dt.bfloat16
U32 = mybir.dt.uint32

_CACHE = {}


def _build_nc():
    nc = bacc.Bacc()
    xT_d = nc.declare_dram_parameter("xT", [D, T], DTB, isOutput=False)
    wqk_d = nc.declare_dram_parameter("wqk", [D, 2 * D], DTB, isOutput=False)
    wv_d = nc.declare_dram_parameter("wvT", [D, D], DTB, isOutput=False)
    wout_d = nc.declare_dram_parameter("woutT", [D, D], DTB, isOutput=False)
    cos_d = nc.declare_dram_parameter("cosT", [128, T], DTB, isOutput=False)
    sin_d = nc.declare_dram_parameter("sinT", [128, T], DTB, isOutput=False)
    bias_d = nc.declare_dram_parameter("bias_rep", [128, D], F32, isOutput=False)
    out_d = nc.declare_dram_parameter("out", [T, D], DTB, isOutput=True)

    MUL = mybir.AluOpType.mult
    ADD = mybir.AluOpType.add
    EXP = mybir.ActivationFunctionType.Exp
    PAIRSWAP = [i ^ 1 for i in range(32)]

    with tile.TileContext(nc) as tc:
        with ExitStack() as ctx:
            consts = ctx.enter_context(tc.tile_pool(name="consts", bufs=1))
            rope = ctx.enter_context(tc.tile_pool(name="rope", bufs=3))
            qkro = ctx.enter_context(tc.tile_pool(name="qkro", bufs=6))
            epool = ctx.enter_context(tc.tile_pool(name="epool", bufs=10))
            orawp = ctx.enter_context(tc.tile_pool(name="orawp", bufs=3))
            zpool = ctx.enter_context(tc.tile_pool(name="zpool", bufs=2))
            ypool = ctx.enter_context(tc.tile_pool(name="ypool", bufs=2))
            # PSUM budget (8 banks): s 2x[128,1024]=4, o 2x[128,512]=2,
            # q (proj/V/rzb shared, atomic units) 2x[128,512]=2.
            ps_s = ctx.enter_context(tc.tile_pool(name="ps_s", bufs=2, space="PSUM"))
            ps_o = ctx.enter_context(tc.tile_pool(name="ps_o", bufs=2, space="PSUM"))
            ps_q = ctx.enter_context(tc.tile_pool(name="ps_q", bufs=2, space="PSUM"))

            # ---- persistent SBUF (per-chunk tensors so matmuls gate on
            # individual DMA chunks, not the whole input load) ----
            xT_c = [consts.tile([128, T], DTB, tag=f"xT{dc}", name=f"xT{dc}")
                    for dc in range(ND)]
            wqkp = [consts.tile([128, ND, 256], DTB, tag=f"wqk{p}", name=f"wqk{p}")
                    for p in range(NP)]
            wv_c = [consts.tile([128, D], DTB, tag=f"wv{dc}", name=f"wv{dc}")
                    for dc in range(ND)]
            wout_s = consts.tile([128, ND, D], DTB, tag="wout")
            cos_s = consts.tile([128, T], DTB, tag="cos")
            sin_s = consts.tile([128, T], DTB, tag="sin")
            bias_s = consts.tile([128, D], F32, tag="bias")
            v_big = consts.tile([128, NT, H, HD + 1], DTB, tag="vbig")
            ocatT = consts.tile([128, NP, T], DTB, tag="ocat")
            selB = consts.tile([HD + 1, HD], DTB, tag="selB")
            dum = consts.tile([1, 16], DTB, tag="dum")
            wz = consts.tile([128, 512], DTB, tag="wz")

            # preload the exp table set on ACT while input DMAs run
            nc.vector.memset(dum, 0.0)
            nc.scalar.activation(dum, dum, EXP)

            nc.vector.memset(wz, 0.0)
            nc.vector.memset(v_big[:, :, :, HD:HD + 1], 1.0)
            nc.vector.memset(selB[HD:HD + 1, :], 1.0)

            # HAM warmup: junk matmuls while the input DMAs stream, so the
            # PE clock gate opens before the real prologue
            warm_ps = ps_s.tile([128, T], F32, tag="s", name="warm")
            for i in range(8):
                nc.tensor.matmul(warm_ps[0:64, 0:512], lhsT=wz[:, 0:64], rhs=wz,
                                 start=True, stop=True)

            # ---- input DMAs, in need-order ----
            xT_r = xT_d[:, :].rearrange("(c p) t -> p c t", p=128)
            wqk_r = wqk_d[:, :].rearrange("(c p) j -> p c j", p=128)
            wv_r = wv_d[:, :].rearrange("(c p) j -> p c j", p=128)
            wout_r = wout_d[:, :].rearrange("(c p) e -> p c e", p=128)
            for dc in range(ND):
                nc.sync.dma_start(out=wqkp[0][:, dc, :], in_=wqk_r[:, dc, 0:256])
                nc.sync.dma_start(out=xT_c[dc], in_=xT_r[:, dc, :])
            nc.sync.dma_start(out=cos_s, in_=cos_d[:, :])
            nc.sync.dma_start(out=sin_s, in_=sin_d[:, :])
            for dc in range(ND):
                nc.sync.dma_start(out=wv_c[dc], in_=wv_r[:, dc, :])
            for p in range(1, NP):
                nc.sync.dma_start(out=wqkp[p],
                                  in_=wqk_r[:, :, 256 * p:256 * (p + 1)])
            for dc in range(ND):
                nc.sync.dma_start(out=wout_s[:, dc, :], in_=wout_r[:, dc, :])
            nc.sync.dma_start(out=bias_s, in_=bias_d[:, :])

            # ---- helpers ----
            ro_q = {}
            ro_k = {}

            def emit_proj_unit(p, w, th):
                """Atomic q-tag unit: one t-half of pair p's q or k projection
                (8 accumulating MMs), PSUM->SBUF cast, RoPE into ro."""
                if p not in ro_q:
                    ro_q[p] = qkro.tile([128, T], DTB, tag="ro", name=f"roq{p}")
                    ro_k[p] = qkro.tile([128, T], DTB, tag="ro", name=f"rok{p}")
                ro = (ro_q if w == "q" else ro_k)[p]
                col0 = 0 if w == "q" else 128
                sl = slice(th * 512, (th + 1) * 512)
                qk_ps = ps_q.tile([128, 512], F32, tag="q", name=f"qkps_{w}{p}_{th}")
                for dc in range(ND):
                    nc.tensor.matmul(
                        qk_ps,
                        lhsT=wqkp[p][:, dc, col0:col0 + 128],
                        rhs=xT_c[dc][:, sl],
                        start=(dc == 0), stop=(dc == ND - 1),
                    )
                raw = rope.tile([128, 512], DTB, tag="raw", name=f"raw_{w}{p}_{th}")
                nc.vector.tensor_copy(raw, qk_ps)
                shuf = rope.tile([128, 512], DTB, tag="shuf", name=f"shuf_{w}{p}_{th}")
                nc.vector.stream_shuffle(shuf.bitcast(U32), raw.bitcast(U32), PAIRSWAP)
                t1 = rope.tile([128, 512], DTB, tag="t1", name=f"t1_{w}{p}_{th}")
                nc.vector.tensor_tensor(t1, shuf, sin_s[:, sl], MUL)
                t2 = rope.tile([128, 512], DTB, tag="t2", name=f"t2_{w}{p}_{th}")
                nc.vector.tensor_tensor(t2, raw, cos_s[:, sl], MUL)
                nc.vector.tensor_tensor(ro[:, sl], t1, t2, ADD)

            def emit_v_unit(tt, jh):
                """Atomic q-tag unit: V projection half-group -> v_big, eviction
                on the scalar engine (ACT has slack early on)."""
                vp = ps_q.tile([128, 512], F32, tag="q", name=f"vps{tt}_{jh}")
                for dc in range(ND):
                    nc.tensor.matmul(
                        vp,
                        lhsT=xT_c[dc][:, tt * 128:(tt + 1) * 128],
                        rhs=wv_c[dc][:, jh * 512:(jh + 1) * 512],
                        start=(dc == 0), stop=(dc == ND - 1),
                    )
                nc.scalar.copy(
                    v_big[:, tt, 8 * jh:8 * (jh + 1), 0:HD],
                    vp.rearrange("q (h v) -> q h v", h=8),
                )

            e_tiles = {}
            o_ps = {}
            zinv = {}

            def emit_qk_exp(pi, jt):
                p, ih = pi // 2, pi % 2
                isl = slice(ih * 512, (ih + 1) * 512)
                s_ps = ps_s.tile([128, T], F32, tag="s", name=f"s{pi}_{jt}")
                for h in range(2):
                    b0 = 64 * h
                    nc.tensor.matmul(
                        s_ps[:, h * 512:(h + 1) * 512],
                        lhsT=ro_k[p][b0:b0 + 64, jt * 128:(jt + 1) * 128],
                        rhs=ro_q[p][b0:b0 + 64, isl],
                        start=True, stop=True,
                        tile_position=(b0, 0),
                    )
                e_t = epool.tile([128, T], DTB, tag="e", name=f"e{pi}_{jt}")
                nc.scalar.activation(e_t, s_ps, EXP, scale=0.125)
                e_tiles[(pi, jt)] = e_t

            def emit_pv(pi, jt):
                p = pi // 2
                if jt == 0:
                    o_ps[pi] = [ps_o.tile([HD + 1, 512], F32, tag="o",
                                          name=f"o{pi}_{h}") for h in range(2)]
                e_t = e_tiles.pop((pi, jt))
                for h in range(2):
                    nc.tensor.matmul(
                        o_ps[pi][h][:, :],
                        lhsT=v_big[:, jt, 2 * p + h, :],
                        rhs=e_t[:, h * 512:(h + 1) * 512],
                        start=(jt == 0), stop=(jt == NT - 1),
                    )
                if jt == NT - 1:
                    # close the pass: evict O (+Z row) to SBUF bf16, freeing
                    # the o psum tiles; normalization happens deferred
                    o_raw = [orawp.tile([HD + 1, 512], DTB, tag="o2",
                                        name=f"o2_{pi}_{h}") for h in range(2)]
                    nc.vector.tensor_copy(o_raw[0], o_ps[pi][0])
                    nc.vector.tensor_copy(o_raw[1], o_ps[pi][1])
                    zinv[pi] = o_raw
                    del o_ps[pi]

            def emit_norm(pi):
                """Atomic q-tag unit: broadcast 1/Z via K=1 matmul per head,
                reciprocal, scale the stashed O rows into ocatT."""
                p, ih = pi // 2, pi % 2
                isl = slice(ih * 512, (ih + 1) * 512)
                o_raw = zinv.pop(pi)
                for h in range(2):
                    rzb = ps_q.tile([HD, 512], F32, tag="q", name=f"rzb{pi}_{h}")
                    nc.tensor.matmul(rzb, lhsT=selB[HD:HD + 1, :],
                                     rhs=o_raw[h][HD:HD + 1, :],
                                     start=True, stop=True)
                    rzb_s = zpool.tile([HD, 512], F32, tag="rzb", name=f"rzbs{pi}_{h}")
                    nc.vector.reciprocal_approx_fast(out=rzb_s, in_=rzb)
                    nc.vector.tensor_tensor(
                        ocatT[64 * h:64 * (h + 1), p, isl],
                        o_raw[h][0:HD, :], rzb_s, MUL)

            # ---- static schedule ----
            # prologue: only the first-half pair-0 projections block slot 0;
            # the second halves are fillers (k th1 needed by slot 4, q th1
            # by slot 8)
            for w, th in (("q", 0), ("k", 0)):
                emit_proj_unit(0, w, th)

            fillers = {s: [] for s in range(128)}
            pv_at = {s: [] for s in range(131)}
            fillers[0].append(lambda: emit_proj_unit(0, "k", 1))
            fillers[1].append(lambda: emit_proj_unit(0, "q", 1))

            # V units: 2 per slot from slot 4
            for k in range(2 * NT):
                tt, jh = k // 2, k % 2
                fillers[4 + k // 2].append(lambda tt=tt, jh=jh: emit_v_unit(tt, jh))
            # pair p>=1 projections: 4 units during the previous pair's slots
            for p in range(1, NP):
                base = 16 * (p - 1) + 6
                for u, (w, th) in enumerate((("q", 0), ("k", 0), ("q", 1), ("k", 1))):
                    fillers[base + 2 * u].append(
                        lambda p=p, w=w, th=th: emit_proj_unit(p, w, th))
            # PV slots: pass0 gated on V availability; pass1 delayed until
            # pass0's o psum closes (o pool has 2 slots); steady lag-2 after
            for pi in range(16):
                for jt in range(NT):
                    if pi == 0:
                        s = 6 + jt
                    elif pi == 1:
                        s = 14 + jt // 2
                    else:
                        s = 8 * pi + jt + 2
                    pv_at[s].append((pi, jt))
            # norm units: a couple slots after each pass closes
            for pi in range(15):
                s = {0: 16, 1: 20}.get(pi, 8 * pi + 12)
                fillers[s].append(lambda pi=pi: emit_norm(pi))

            # ---- the flat slot stream ----
            for s in range(128):
                emit_qk_exp(s // 8, s % 8)
                for pi, jt in pv_at[s]:
                    emit_pv(pi, jt)
                for f in fillers[s]:
                    f()
            for s in range(128, 131):
                for pi, jt in pv_at[s]:
                    emit_pv(pi, jt)
            emit_norm(15)

            # ---- output projection: y = ocatT^T @ woutT + bias ----
            for tt in range(NT):
                y_ps = ps_s.tile([128, D], F32, tag="s", name=f"yps{tt}")
                for fc in range(NP):
                    for eh in range(2):
                        nc.tensor.matmul(
                            y_ps[:, eh * 512:(eh + 1) * 512],
                            lhsT=ocatT[:, fc, tt * 128:(tt + 1) * 128],
                            rhs=wout_s[:, fc, eh * 512:(eh + 1) * 512],
                            start=(fc == 0), stop=(fc == NP - 1),
                        )
                y_t = ypool.tile([128, D], DTB, tag="y", name=f"y{tt}")
                nc.vector.tensor_tensor(y_t, y_ps, bias_s, ADD)
                nc.sync.dma_start(out=out_d[tt * 128:(tt + 1) * 128, :], in_=y_t)

    nc.compile()
    return nc


def _rope_tables():
    inv_freq = 1.0 / (THETA ** (np.arange(0, HD, 2, dtype=np.float64) / HD))  # [32]
    t = np.arange(T, dtype=np.float64)
    freqs = t[:, None] * inv_freq[None, :]            # [T, 32]
    emb = np.repeat(freqs, 2, axis=-1)                # [T, 64]
    cos_dt = np.cos(emb).T.astype(np.float32)         # [64, T]
    sin_dt = np.sin(emb).T.astype(np.float32)
    sign = np.where(np.arange(HD) % 2 == 0, -1.0, 1.0).astype(np.float32)
    sin_signed = sin_dt * sign[:, None]
    cosT = np.tile(cos_dt, (2, 1)).astype(BF16)       # [128, T]
    sinT = np.tile(sin_signed, (2, 1)).astype(BF16)
    return cosT, sinT


def get_nc():
    if "nc" not in _CACHE:
        _CACHE["nc"] = _build_nc()
    return _CACHE["nc"]


def make_in_maps(x, mask, Wqkv, Wout, bout):
    cosT, sinT = _rope_tables()
    wqkvT = np.ascontiguousarray(np.asarray(Wqkv, dtype=np.float32).T)  # [d, 3D]
    # pair-major q|k column blocks: [d, 16*128]
    blocks = []
    for p in range(NP):
        blocks.append(wqkvT[:, 128 * p:128 * (p + 1)])
        blocks.append(wqkvT[:, D + 128 * p:D + 128 * (p + 1)])
    wqk = np.ascontiguousarray(np.concatenate(blocks, axis=1)).astype(BF16)
    wvT = np.ascontiguousarray(wqkvT[:, 2 * D:3 * D]).astype(BF16)
    woutT = np.ascontiguousarray(np.asarray(Wout, dtype=np.float32).T).astype(BF16)
    bias_rep = np.tile(np.asarray(bout, dtype=np.float32)[None, :], (128, 1))
    x = np.asarray(x, dtype=np.float32)
    in_maps = []
    for c in range(B):
        xT = np.ascontiguousarray(x[c].T).astype(BF16)
        in_maps.append({
            "xT": xT, "wqk": wqk, "wvT": wvT, "woutT": woutT,
            "cosT": cosT, "sinT": sinT, "bias_rep": bias_rep,
        })
    return in_maps


LAST_EXEC_NS = None


def kernel(x, mask, Wqkv, Wout, bout):
    global LAST_EXEC_NS
    from concourse.bass_utils import run_bass_kernel_spmd

    nc = get_nc()
    in_maps = make_in_maps(x, mask, Wqkv, Wout, bout)
    trace = bool(os.environ.get("BASS_TRACE"))
    res = run_bass_kernel_spmd(nc, in_maps, core_ids=list(range(B)), trace=trace)
    LAST_EXEC_NS = res.exec_time_ns
    out = np.stack([np.asarray(res.results[c]["out"]) for c in range(B)], axis=0)
    return out.astype(np.float32)
